# revision 12
# baseline (speedup 1.0000x reference)
"""Fused cross-modal attention (concat two QKV streams along sequence, full
softmax attention) on 8 Trainium2 NeuronCores.

Sharding: data-parallel over (batch b, query-half h) -> 8 shards. Each core
computes attention for 2048 queries against the fused 4096-key sequence.

Per-core kernel (all operands resident in SBUF):
  - Q, K, V loaded with fully-contiguous DMA using a "(p t) d -> p (t d)"
    layout (each partition gets a contiguous HBM span). This permutes the
    key order (softmax-invariant as long as K and V share it) and the query
    order (inverted on the host when assembling the output).
  - Q and K tiles are PE-transposed to d-major (qt [64, 2048], kt [64, 4096]).
  - Main loop over 2 query-halves x 32 key-tiles:
      scoresT[k=128, q=1024]  = kt_chunk.T @ qt_block      (PSUM, 2 matmuls)
      expS = exp(scale*scoresT)                            (one ACT instr)
      acc[65, 1024]          += [V_chunk | 1].T @ expS     (PSUM accumulate)
    The ones-column appended to V yields the softmax denominator in
    partition 64 of the accumulator for free.
  - Normalize: broadcast denominator row across partitions (SWDGE),
    reciprocal + multiply on DVE, DMA out as outT [64, 2048].
Host assembles [4, 4096, 64] and inverts the query permutation.
"""

import numpy as np

import concourse.bass as bass
import concourse.tile as tile
from concourse import mybir
from concourse.bacc import Bacc
from concourse.bass_utils import run_bass_kernel_spmd
from concourse.masks import make_identity

F32 = mybir.dt.float32

B, S, D = 4, 2048, 64
S2 = 2 * S  # fused sequence length 4096
NCORES = 8
QSH = 2048  # queries per core (= S: half of the fused sequence)
KT = S2 // 128  # 32 key tiles of 128
QT = QSH // 128  # 16 query tiles of 128
SCALE = 1.0 / float(np.sqrt(D))

# Column c of the on-chip output corresponds to query (c % 128) * (QSH // 128)
# + (c // 128) of the shard (from the "(p t) d -> p (t d)" load layout).
_COL2QUERY = (np.arange(QSH) % 128) * QT + (np.arange(QSH) // 128)


def _build():
    nc = Bacc()
    q = nc.declare_dram_parameter("q", [QSH, D], F32, isOutput=False)
    k = nc.declare_dram_parameter("k", [S2, D], F32, isOutput=False)
    v = nc.declare_dram_parameter("v", [S2, D], F32, isOutput=False)
    out = nc.declare_dram_parameter("out", [D, QSH], F32, isOutput=True)

    with tile.TileContext(nc) as tc:
        with (
            tc.tile_pool(name="const", bufs=1) as const_pool,
            tc.tile_pool(name="stage", bufs=1) as stage,
            tc.tile_pool(name="tpsum", bufs=2, space="PSUM") as tpsum,
            tc.tile_pool(name="spsum", bufs=2, space="PSUM") as spsum,
            tc.tile_pool(name="apsum", bufs=1, space="PSUM") as apsum,
            tc.tile_pool(name="exps", bufs=3) as exps,
            tc.tile_pool(name="outp", bufs=2) as outp,
            tc.tile_pool(name="dram", bufs=2, space="DRAM") as dram,
        ):
            ident = const_pool.tile([128, 128], F32)
            make_identity(nc, ident)

            q_ap = q[:].rearrange("(p t) d -> p (t d)", p=128)  # [128, 16*64]
            k_ap = k[:].rearrange("(p t) d -> p (t d)", p=128)  # [128, 32*64]
            v_ap = v[:].rearrange("(p t) d -> p (t d)", p=128)

            # Contiguous staging loads, one tile per DMA chunk so each
            # consumer depends on exactly one DMA completion.
            NKC = 8  # tiles per k/v load chunk
            NQC = 8
            k_chunks = []
            for c0 in range(0, KT, NKC):
                t = stage.tile([128, NKC, D], F32, tag=f"k_nat{c0}")
                nc.sync.dma_start(out=t, in_=k_ap[:, c0 * D : (c0 + NKC) * D])
                k_chunks.append(t)
            q_chunks = []
            for c0 in range(0, QT, NQC):
                t = stage.tile([128, NQC, D], F32, tag=f"q_nat{c0}")
                nc.sync.dma_start(out=t, in_=q_ap[:, c0 * D : (c0 + NQC) * D])
                q_chunks.append(t)
            v_chunks = []
            for c0 in range(0, KT, NKC):
                t = stage.tile([128, NKC, D], F32, tag=f"v_nat{c0}")
                nc.sync.dma_start(out=t, in_=v_ap[:, c0 * D : (c0 + NKC) * D])
                v_chunks.append(t)

            # [V | 1] weight tiles for the PV matmul.
            v1 = stage.tile([128, KT, D + 1], F32)
            for ci, vc in enumerate(v_chunks):
                nc.vector.tensor_copy(
                    out=v1[:, ci * NKC : (ci + 1) * NKC, 0:D], in_=vc
                )
            nc.vector.memset(v1[:, :, D : D + 1], 1.0)

            # d-major transposed copies of Q and K.
            qt = stage.tile([64, QT * 128], F32)
            kt = stage.tile([64, KT * 128], F32)

            def transpose_k(i):
                pt = tpsum.tile([64, 128], F32, tag="tp")
                nc.tensor.transpose(pt, k_chunks[i // NKC][:, i % NKC, :], ident)
                nc.vector.tensor_copy(out=kt[:, i * 128 : (i + 1) * 128], in_=pt)

            def transpose_q(i):
                pt = tpsum.tile([64, 128], F32, tag="tp")
                nc.tensor.transpose(pt, q_chunks[i // NQC][:, i % NQC, :], ident)
                nc.vector.tensor_copy(out=qt[:, i * 128 : (i + 1) * 128], in_=pt)

            # Emit enough transposes up front for the first loop iterations;
            # the rest are interleaved into the main loop emission order.
            transpose_k(0)
            for i in range(QT):
                transpose_q(i)
            pending_k = list(range(1, KT))

            for h in range(2):  # query halves of 1024 columns
                acc = apsum.tile([65, 1024], F32)
                for i in range(KT):
                    sc = spsum.tile([128, 1024], F32)
                    for j in range(2):
                        nc.tensor.matmul(
                            sc[:, j * 512 : (j + 1) * 512],
                            lhsT=kt[:, i * 128 : (i + 1) * 128],
                            rhs=qt[:, h * 1024 + j * 512 : h * 1024 + (j + 1) * 512],
                            start=True,
                            stop=True,
                        )
                    ex = exps.tile([128, 1024], F32)
                    nc.scalar.activation(
                        out=ex,
                        in_=sc,
                        func=mybir.ActivationFunctionType.Exp,
                        scale=SCALE,
                    )
                    for j in range(2):
                        nc.tensor.matmul(
                            acc[:, j * 512 : (j + 1) * 512],
                            lhsT=v1[:, i, :],
                            rhs=ex[:, j * 512 : (j + 1) * 512],
                            start=(i == 0),
                            stop=(i == KT - 1),
                            skip_group_check=True,
                        )
                    if pending_k:
                        transpose_k(pending_k.pop(0))

                # Normalize: out[:, q] = acc[0:64, q] / acc[64, q].
                # DMA-broadcast the PSUM denominator row across 64 SBUF
                # partitions (stride-0 partition read; DMA is a crossbar).
                den_sb = outp.tile([65, 1024], F32, tag="den_sb")
                nc.vector.tensor_copy(out=den_sb[64:65, :], in_=acc[64:65, :])
                # Broadcast across partitions via a DRAM bounce: engines
                # can't read across partitions, but a DRAM-source DMA with a
                # stride-0 partition AP can fan one row out to 64 partitions.
                den_dram = dram.tile([1, 1024], F32, tag="den_dram")
                nc.sync.dma_start(out=den_dram, in_=den_sb[64:65, :])
                den_b = outp.tile([64, 1024], F32, tag="den")
                nc.gpsimd.dma_start(
                    out=den_b, in_=den_dram[0:1, :].to_broadcast([64, 1024])
                )
                rec = outp.tile([64, 1024], F32, tag="rec")
                nc.vector.reciprocal(rec, den_b)
                ot = outp.tile([64, 1024], F32, tag="ot")
                nc.vector.tensor_mul(ot, acc[0:64, :], rec)
                nc.sync.dma_start(
                    out=out[:][:, h * 1024 : (h + 1) * 1024], in_=ot
                )

    nc.finalize()
    return nc


_NC = None


def _get_nc():
    global _NC
    if _NC is None:
        _NC = _build()
    return _NC


def _shard_inputs(Q1, K1, V1, Q2, K2, V2):
    """Core c handles batch c//2, query-half c%2."""
    in_maps = []
    for c in range(NCORES):
        b, h = divmod(c, 2)
        qs = Q1[b] if h == 0 else Q2[b]
        ks = np.concatenate([K1[b], K2[b]], axis=0)
        vs = np.concatenate([V1[b], V2[b]], axis=0)
        in_maps.append(
            {
                "q": np.ascontiguousarray(qs, dtype=np.float32),
                "k": np.ascontiguousarray(ks, dtype=np.float32),
                "v": np.ascontiguousarray(vs, dtype=np.float32),
            }
        )
    return in_maps


def _assemble(results):
    out = np.empty((B, S2, D), dtype=np.float32)
    for c in range(NCORES):
        b, h = divmod(c, 2)
        shard = np.empty((QSH, D), dtype=np.float32)
        shard[_COL2QUERY, :] = results[c]["out"].T
        out[b, h * QSH : (h + 1) * QSH, :] = shard
    return out


def run(inputs, trace=False):
    nc = _get_nc()
    in_maps = _shard_inputs(
        np.asarray(inputs["Q1"]), np.asarray(inputs["K1"]), np.asarray(inputs["V1"]),
        np.asarray(inputs["Q2"]), np.asarray(inputs["K2"]), np.asarray(inputs["V2"]),
    )
    bkr = run_bass_kernel_spmd(nc, in_maps, list(range(NCORES)), trace=trace)
    return _assemble(bkr.results), bkr


def kernel(**inputs) -> np.ndarray:
    out, _ = run(inputs)
    return out


# revision 19
# speedup vs baseline: 1.6367x; 1.6367x over previous
"""Fused cross-modal attention (concat two QKV streams along sequence, full
softmax attention) on 8 Trainium2 NeuronCores.

Sharding: data-parallel over (batch b, query-half h) -> 8 shards. Each core
computes attention for 2048 queries against the fused 4096-key sequence.

Per-core kernel (all operands resident in SBUF):
  - Q, K, V loaded with fully-contiguous DMA using a "(p t) d -> p (t d)"
    layout (each partition gets a contiguous HBM span). This permutes the
    key order (softmax-invariant: K and V share the permutation) and the
    query order (undone by the output DMA access pattern).
  - Q and K tiles are PE-transposed to d-major (qt [64, 2048], kt [64, 4096]).
  - Main loop over 2 query-halves x 32 key-tiles:
      scoresT[k=128, q=1024]  = kt_chunk.T @ qt_block     (PSUM, 2 matmuls)
      expS = exp(scale*scoresT)                           (one ACT instr)
      acc[65, 1024]          += [V_chunk | 1].T @ expS    (PSUM accumulate)
    Matmul operands are bitcast to float32r: full 4-byte fp32 data pushed
    through the PE in replication mode, 1 cycle/row at N=512 instead of
    fp32's 2 half-rate passes (4 cycles/row).
    The ones-column appended to V yields the softmax denominator in
    partition 64 of the accumulator for free.
  - Epilogue per query-half: copy acc to SBUF (frees PSUM for the next
    half), PE-transpose 128-query chunks to [128, 65], then per-partition
    reciprocal of the denominator column + tensor_scalar multiply, and DMA
    straight to the [2048, 64] output with a stride-16-row access pattern
    that undoes the query permutation.
"""

import numpy as np

import concourse.bass as bass
import concourse.tile as tile
from concourse import mybir
from concourse.bacc import Bacc
from concourse.bass_utils import run_bass_kernel_spmd
from concourse.masks import make_identity

F32 = mybir.dt.float32
F32R = mybir.dt.float32r

B, S, D = 4, 2048, 64
S2 = 2 * S  # fused sequence length 4096
NCORES = 8
QSH = 2048  # queries per core (= S: half of the fused sequence)
KT = S2 // 128  # 32 key tiles of 128
QT = QSH // 128  # 16 query tiles of 128
SCALE = 1.0 / float(np.sqrt(D))


def _build():
    nc = Bacc()
    q = nc.declare_dram_parameter("q", [QSH, D], F32, isOutput=False)
    k = nc.declare_dram_parameter("k", [S2, D], F32, isOutput=False)
    v = nc.declare_dram_parameter("v", [S2, D], F32, isOutput=False)
    out = nc.declare_dram_parameter("out", [QSH, D], F32, isOutput=True)

    with tile.TileContext(nc) as tc:
        with (
            tc.tile_pool(name="const", bufs=1) as const_pool,
            tc.tile_pool(name="stage", bufs=1) as stage,
            tc.tile_pool(name="psum", bufs=2, space="PSUM") as psum,
            tc.tile_pool(name="apsum", bufs=1, space="PSUM") as apsum,
            tc.tile_pool(name="exps", bufs=3) as exps,
            tc.tile_pool(name="outp", bufs=3) as outp,
        ):
            ident = const_pool.tile([128, 128], F32)
            make_identity(nc, ident)

            q_ap = q[:].rearrange("(p t) d -> p (t d)", p=128)  # [128, 16*64]
            k_ap = k[:].rearrange("(p t) d -> p (t d)", p=128)  # [128, 32*64]
            v_ap = v[:].rearrange("(p t) d -> p (t d)", p=128)
            # out[p*16 + t, :] <- chunk t, partition p
            out_ap = out[:].rearrange("(p t) d -> t p d", t=QT)

            # Contiguous staging loads, one tile per DMA chunk so each
            # consumer depends on exactly one DMA completion.
            NKC = 8  # tiles per k/v load chunk
            NQC = 8
            k_chunks = []
            for c0 in range(0, KT, NKC):
                t = stage.tile([128, NKC, D], F32, tag=f"k_nat{c0}")
                nc.sync.dma_start(out=t, in_=k_ap[:, c0 * D : (c0 + NKC) * D])
                k_chunks.append(t)
            q_chunks = []
            for c0 in range(0, QT, NQC):
                t = stage.tile([128, NQC, D], F32, tag=f"q_nat{c0}")
                nc.sync.dma_start(out=t, in_=q_ap[:, c0 * D : (c0 + NQC) * D])
                q_chunks.append(t)
            v_chunks = []
            for c0 in range(0, KT, NKC):
                t = stage.tile([128, NKC, D], F32, tag=f"v_nat{c0}")
                nc.sync.dma_start(out=t, in_=v_ap[:, c0 * D : (c0 + NKC) * D])
                v_chunks.append(t)

            # [V | 1] weight tiles for the PV matmul.
            v1 = stage.tile([128, KT, D + 1], F32R)
            for ci, vc in enumerate(v_chunks):
                nc.vector.tensor_copy(
                    out=v1[:, ci * NKC : (ci + 1) * NKC, 0:D], in_=vc
                )
            ones_f32 = stage.tile([128, KT], F32, tag="ones")
            nc.vector.memset(ones_f32, 1.0)
            nc.vector.tensor_copy(out=v1[:, :, D], in_=ones_f32)

            # d-major transposed copies of Q and K.
            qt = stage.tile([64, QT * 128], F32R)
            kt = stage.tile([64, KT * 128], F32R)

            def transpose_k(i):
                pt = psum.tile([128, 128], F32, tag="tp")
                nc.tensor.transpose(
                    pt[0:64, :], k_chunks[i // NKC][:, i % NKC, :], ident
                )
                nc.vector.tensor_copy(
                    out=kt[:, i * 128 : (i + 1) * 128], in_=pt[0:64, :]
                )

            def transpose_q(i):
                pt = psum.tile([128, 128], F32, tag="tp")
                nc.tensor.transpose(
                    pt[0:64, :], q_chunks[i // NQC][:, i % NQC, :], ident
                )
                nc.vector.tensor_copy(
                    out=qt[:, i * 128 : (i + 1) * 128], in_=pt[0:64, :]
                )

            transpose_k(0)
            for i in range(NQC):
                transpose_q(i)
            for i in range(1, KT):
                transpose_k(i)
            for i in range(NQC, QT):
                transpose_q(i)

            for h in range(2):  # query halves of 1024 columns
                acc = apsum.tile([65, 1024], F32)
                for i in range(KT):
                    sc = psum.tile([128, 1024], F32, tag="sc")
                    for j in range(2):
                        nc.tensor.matmul(
                            sc[:, j * 512 : (j + 1) * 512],
                            lhsT=kt[:, i * 128 : (i + 1) * 128],
                            rhs=qt[
                                :, h * 1024 + j * 512 : h * 1024 + (j + 1) * 512
                            ],
                            start=True,
                            stop=True,
                        )
                    ex = exps.tile([128, 1024], F32R)
                    nc.scalar.activation(
                        out=ex,
                        in_=sc,
                        func=mybir.ActivationFunctionType.Exp,
                        scale=SCALE,
                    )
                    for j in range(2):
                        nc.tensor.matmul(
                            acc[:, j * 512 : (j + 1) * 512],
                            lhsT=v1[:, i, :],
                            rhs=ex[:, j * 512 : (j + 1) * 512],
                            start=(i == 0),
                            stop=(i == KT - 1),
                            skip_group_check=True,
                        )


                # Epilogue: copy acc out of PSUM (frees it for the next
                # half), then transpose + normalize 128-query chunks.
                acc_sb = outp.tile([65, 1024], F32, tag="acc_sb")
                nc.vector.tensor_copy(out=acc_sb, in_=acc)
                for t in range(8):
                    it = h * 8 + t  # global query-tile index
                    tr = psum.tile([128, 128], F32, tag="tp")
                    nc.tensor.transpose(
                        tr[:, 0:65],
                        acc_sb[:, t * 128 : (t + 1) * 128],
                        ident[0:65, 0:65],
                    )
                    rc = outp.tile([128, 1], F32, tag="rc")
                    nc.vector.reciprocal(rc, tr[:, 64:65])
                    ot = outp.tile([128, D], F32, tag="ot")
                    nc.vector.tensor_scalar_mul(ot, tr[:, 0:D], rc)
                    nc.sync.dma_start(out=out_ap[it], in_=ot)

    nc.finalize()
    return nc


_NC = None


def _get_nc():
    global _NC
    if _NC is None:
        _NC = _build()
    return _NC


def _shard_inputs(Q1, K1, V1, Q2, K2, V2):
    """Core c handles batch c//2, query-half c%2."""
    in_maps = []
    for c in range(NCORES):
        b, h = divmod(c, 2)
        qs = Q1[b] if h == 0 else Q2[b]
        ks = np.concatenate([K1[b], K2[b]], axis=0)
        vs = np.concatenate([V1[b], V2[b]], axis=0)
        in_maps.append(
            {
                "q": np.ascontiguousarray(qs, dtype=np.float32),
                "k": np.ascontiguousarray(ks, dtype=np.float32),
                "v": np.ascontiguousarray(vs, dtype=np.float32),
            }
        )
    return in_maps


def _assemble(results):
    out = np.empty((B, S2, D), dtype=np.float32)
    for c in range(NCORES):
        b, h = divmod(c, 2)
        out[b, h * QSH : (h + 1) * QSH, :] = results[c]["out"]
    return out


def run(inputs, trace=False):
    nc = _get_nc()
    in_maps = _shard_inputs(
        np.asarray(inputs["Q1"]), np.asarray(inputs["K1"]), np.asarray(inputs["V1"]),
        np.asarray(inputs["Q2"]), np.asarray(inputs["K2"]), np.asarray(inputs["V2"]),
    )
    bkr = run_bass_kernel_spmd(nc, in_maps, list(range(NCORES)), trace=trace)
    return _assemble(bkr.results), bkr


def kernel(**inputs) -> np.ndarray:
    out, _ = run(inputs)
    return out


# revision 23
# speedup vs baseline: 1.7969x; 1.0979x over previous
"""Fused cross-modal attention (concat two QKV streams along sequence, full
softmax attention) on 8 Trainium2 NeuronCores.

Sharding: data-parallel over (batch b, query-half h) -> 8 shards. Each core
computes attention for 2048 queries against the fused 4096-key sequence.

Per-core kernel (all operands resident in SBUF):
  - Q, K, V loaded with fully-contiguous DMA using a "(p t) d -> p (t d)"
    layout (each partition gets a contiguous HBM span). This permutes the
    key order (softmax-invariant: K and V share the permutation) and the
    query order (undone by the output DMA access pattern).
  - Q and K tiles are PE-transposed to d-major (qt [64, 2048], kt [64, 4096]).
  - Main loop over 2 query-halves x 32 key-tiles:
      scoresT[k=128, q=1024]  = kt_chunk.T @ qt_block     (PSUM, 2 matmuls)
      expS = exp(scale*scoresT)                           (one ACT instr)
      acc[65, 1024]          += [V_chunk | 1].T @ expS    (PSUM accumulate)
    Matmul operands are bitcast to float32r: full 4-byte fp32 data pushed
    through the PE in replication mode, 1 cycle/row at N=512 instead of
    fp32's 2 half-rate passes (4 cycles/row).
    The ones-column appended to V yields the softmax denominator in
    partition 64 of the accumulator for free.
  - Epilogue per query-half: copy acc to SBUF (frees PSUM for the next
    half), PE-transpose 128-query chunks to [128, 65], then per-partition
    reciprocal of the denominator column + tensor_scalar multiply, and DMA
    straight to the [2048, 64] output with a stride-16-row access pattern
    that undoes the query permutation.
"""

import numpy as np

import concourse.bass as bass
import concourse.tile as tile
from concourse import mybir
from concourse.bacc import Bacc
from concourse.bass_utils import run_bass_kernel_spmd
from concourse.masks import make_identity

F32 = mybir.dt.float32
F32R = mybir.dt.float32r

B, S, D = 4, 2048, 64
S2 = 2 * S  # fused sequence length 4096
NCORES = 8
QSH = 2048  # queries per core (= S: half of the fused sequence)
KT = S2 // 128  # 32 key tiles of 128
QT = QSH // 128  # 16 query tiles of 128
SCALE = 1.0 / float(np.sqrt(D))


def _build():
    nc = Bacc()
    q = nc.declare_dram_parameter("q", [QSH, D], F32, isOutput=False)
    k = nc.declare_dram_parameter("k", [S2, D], F32, isOutput=False)
    v = nc.declare_dram_parameter("v", [S2, D], F32, isOutput=False)
    out = nc.declare_dram_parameter("out", [QSH, D], F32, isOutput=True)

    with tile.TileContext(nc) as tc:
        with (
            tc.tile_pool(name="const", bufs=1) as const_pool,
            tc.tile_pool(name="stage", bufs=1) as stage,
            tc.tile_pool(name="psum", bufs=2, space="PSUM") as psum,
            tc.tile_pool(name="apsum", bufs=1, space="PSUM") as apsum,
            tc.tile_pool(name="exps", bufs=3) as exps,
            tc.tile_pool(name="outp", bufs=3) as outp,
        ):
            ident = const_pool.tile([128, 128], F32)
            make_identity(nc, ident)
            # Touch Exp early so the ~2.7us ACT table load overlaps the
            # input DMAs instead of stalling the first real exp.
            warm = const_pool.tile([128, 1], F32)
            nc.scalar.activation(
                out=warm, in_=ident[:, 0:1],
                func=mybir.ActivationFunctionType.Exp,
            )

            q_ap = q[:].rearrange("(p t) d -> p (t d)", p=128)  # [128, 16*64]
            k_ap = k[:].rearrange("(p t) d -> p (t d)", p=128)  # [128, 32*64]
            v_ap = v[:].rearrange("(p t) d -> p (t d)", p=128)
            # out[p*16 + h*8 + t, :] <- half h, chunk t, partition p;
            # the 8 chunks of one half are contiguous rows per partition.
            out_ap = out[:].rearrange("(p g t) d -> g p (t d)", g=2, t=8)

            # Contiguous staging loads, one tile per DMA chunk so each
            # consumer depends on exactly one DMA completion.
            NKC = 8  # tiles per k/v load chunk
            NQC = 8
            k_chunks = []
            for c0 in range(0, KT, NKC):
                t = stage.tile([128, NKC, D], F32, tag=f"k_nat{c0}")
                nc.sync.dma_start(out=t, in_=k_ap[:, c0 * D : (c0 + NKC) * D])
                k_chunks.append(t)
            q_chunks = []
            for c0 in range(0, QT, NQC):
                t = stage.tile([128, NQC, D], F32, tag=f"q_nat{c0}")
                nc.sync.dma_start(out=t, in_=q_ap[:, c0 * D : (c0 + NQC) * D])
                q_chunks.append(t)
            v_chunks = []
            for c0 in range(0, KT, NKC):
                t = stage.tile([128, NKC, D], F32, tag=f"v_nat{c0}")
                nc.sync.dma_start(out=t, in_=v_ap[:, c0 * D : (c0 + NKC) * D])
                v_chunks.append(t)

            # [V | 1] weight tiles for the PV matmul.
            v1 = stage.tile([128, KT, D + 1], F32R)
            for ci, vc in enumerate(v_chunks):
                nc.vector.tensor_copy(
                    out=v1[:, ci * NKC : (ci + 1) * NKC, 0:D], in_=vc
                )
            ones_f32 = stage.tile([128, KT], F32, tag="ones")
            nc.vector.memset(ones_f32, 1.0)
            nc.vector.tensor_copy(out=v1[:, :, D], in_=ones_f32)

            # d-major transposed copies of Q and K.
            qt = stage.tile([64, QT * 128], mybir.dt.bfloat16)
            kt = stage.tile([64, KT * 128], mybir.dt.bfloat16)

            def transpose_k(i):
                pt = psum.tile([128, 128], F32, tag="tp")
                nc.tensor.transpose(
                    pt[0:64, :], k_chunks[i // NKC][:, i % NKC, :], ident
                )
                nc.vector.tensor_copy(
                    out=kt[:, i * 128 : (i + 1) * 128], in_=pt[0:64, :]
                )

            def transpose_q(i):
                pt = psum.tile([128, 128], F32, tag="tp")
                nc.tensor.transpose(
                    pt[0:64, :], q_chunks[i // NQC][:, i % NQC, :], ident
                )
                nc.vector.tensor_copy(
                    out=qt[:, i * 128 : (i + 1) * 128], in_=pt[0:64, :]
                )

            transpose_k(0)
            for i in range(NQC):
                transpose_q(i)
            for i in range(1, KT):
                transpose_k(i)
            for i in range(NQC, QT):
                transpose_q(i)

            for h in range(2):  # query halves of 1024 columns
                acc = apsum.tile([65, 1024], F32)
                for i in range(KT):
                    sc = psum.tile([128, 1024], F32, tag="sc")
                    for j in range(2):
                        nc.tensor.matmul(
                            sc[:, j * 512 : (j + 1) * 512],
                            lhsT=kt[:, i * 128 : (i + 1) * 128],
                            rhs=qt[
                                :, h * 1024 + j * 512 : h * 1024 + (j + 1) * 512
                            ],
                            start=True,
                            stop=True,
                        )
                    ex = exps.tile([128, 1024], F32R)
                    nc.scalar.activation(
                        out=ex,
                        in_=sc,
                        func=mybir.ActivationFunctionType.Exp,
                        scale=SCALE,
                    )
                    for j in range(2):
                        nc.tensor.matmul(
                            acc[:, j * 512 : (j + 1) * 512],
                            lhsT=v1[:, i, :],
                            rhs=ex[:, j * 512 : (j + 1) * 512],
                            start=(i == 0),
                            stop=(i == KT - 1),
                            skip_group_check=True,
                        )


                # Epilogue: copy acc out of PSUM (frees it for the next
                # half), then transpose + normalize 128-query chunks.
                acc_sb = outp.tile([65, 1024], F32, tag="acc_sb")
                nc.vector.tensor_copy(out=acc_sb, in_=acc)
                ot = outp.tile([128, 8, D], F32, tag="ot")
                for t in range(8):
                    tr = psum.tile([128, 128], F32, tag="tp")
                    nc.tensor.transpose(
                        tr[:, 0:65],
                        acc_sb[:, t * 128 : (t + 1) * 128],
                        ident[0:65, 0:65],
                    )
                    rc = outp.tile([128, 1], F32, tag="rc")
                    nc.vector.reciprocal(rc, tr[:, 64:65])
                    nc.vector.tensor_scalar_mul(ot[:, t, :], tr[:, 0:D], rc)
                nc.sync.dma_start(out=out_ap[h], in_=ot)

    nc.finalize()
    return nc


_NC = None


def _get_nc():
    global _NC
    if _NC is None:
        _NC = _build()
    return _NC


def _shard_inputs(Q1, K1, V1, Q2, K2, V2):
    """Core c handles batch c//2, query-half c%2."""
    in_maps = []
    for c in range(NCORES):
        b, h = divmod(c, 2)
        qs = Q1[b] if h == 0 else Q2[b]
        ks = np.concatenate([K1[b], K2[b]], axis=0)
        vs = np.concatenate([V1[b], V2[b]], axis=0)
        in_maps.append(
            {
                "q": np.ascontiguousarray(qs, dtype=np.float32),
                "k": np.ascontiguousarray(ks, dtype=np.float32),
                "v": np.ascontiguousarray(vs, dtype=np.float32),
            }
        )
    return in_maps


def _assemble(results):
    out = np.empty((B, S2, D), dtype=np.float32)
    for c in range(NCORES):
        b, h = divmod(c, 2)
        out[b, h * QSH : (h + 1) * QSH, :] = results[c]["out"]
    return out


def run(inputs, trace=False):
    nc = _get_nc()
    in_maps = _shard_inputs(
        np.asarray(inputs["Q1"]), np.asarray(inputs["K1"]), np.asarray(inputs["V1"]),
        np.asarray(inputs["Q2"]), np.asarray(inputs["K2"]), np.asarray(inputs["V2"]),
    )
    bkr = run_bass_kernel_spmd(nc, in_maps, list(range(NCORES)), trace=trace)
    return _assemble(bkr.results), bkr


def kernel(**inputs) -> np.ndarray:
    out, _ = run(inputs)
    return out
